# revision 5
# baseline (speedup 1.0000x reference)
"""Distributed Trainium2 Bass kernel: RMSNorm + 16-head attention + out-proj.

Problem (hardcoded): tokens [4, 2048, 2048] f32, DIM=2048, HEADS=16, DHEAD=128.
Sharding: 8 cores = 4 batches x 2 sequence halves. Each core owns 1024 query
rows of one batch; K/V for the full 2048 keys arrive via a pair-wise AllGather
(cores 2i and 2i+1 hold the two halves of batch i). All matmuls run in bf16
with fp32 PSUM accumulation; rmsnorm is computed in fp32.

Host-side folding: norm_weight and the q-scale (dhead^-0.5) are folded into
the projection weight matrices, so the device kernel only computes
x * rsqrt(mean(x^2) + eps) for the norm.
"""

import os
import sys

for p in ("/opt/trn_rl_repo", "/root/.axon_site/_ro/trn_rl_repo"):
    if os.path.isdir(p) and p not in sys.path:
        sys.path.append(p)

import numpy as np
import ml_dtypes

BF16 = ml_dtypes.bfloat16

DIM = 2048
HEADS = 16
DHEAD = 128
B = 4
N = 2048
NCORES = 8
LOCAL = N // 2          # 1024 rows per core
P = 128                 # partitions
MC = DIM // P           # 16 model chunks
RT = LOCAL // P         # 8 row tiles
QH = 2                  # qrow halves of 512
FD = 512                # moving free dim
KCHUNKS = N // P        # 16 key chunks (over both blocks)
OC = DIM // FD          # 4 out-col chunks
EPS = 1.1920929e-07

_CACHED_NC = None


def build():
    from concourse import bacc, tile, mybir
    from concourse import bass

    fp32 = mybir.dt.float32
    bf16 = mybir.dt.bfloat16

    nc = bacc.Bacc(
        "TRN2", target_bir_lowering=False, debug=False, num_devices=NCORES
    )

    toks = nc.dram_tensor("tokens", [LOCAL, DIM], fp32, kind="ExternalInput")
    wq = nc.dram_tensor("wq", [HEADS, P, MC, P], bf16, kind="ExternalInput")
    wk = nc.dram_tensor("wk", [HEADS, P, MC, P], bf16, kind="ExternalInput")
    wv = nc.dram_tensor("wv", [DIM, DIM], bf16, kind="ExternalInput")
    wo = nc.dram_tensor("wo", [DIM, DIM], bf16, kind="ExternalInput")
    out = nc.dram_tensor("out", [LOCAL, DIM], fp32, kind="ExternalOutput")

    # internal DRAM scratch
    kl = nc.dram_tensor("kl", [HEADS, P, LOCAL], bf16)        # local k^T per head
    vl = nc.dram_tensor("vl", [RT, P, DIM], bf16)             # local v (natural)
    kg = nc.dram_tensor("kg", [2, HEADS, P, LOCAL], bf16)     # gathered k^T
    vg = nc.dram_tensor("vg", [2, RT, P, DIM], bf16)          # gathered v

    RG = [[0, 1], [2, 3], [4, 5], [6, 7]]

    with tile.TileContext(nc) as tc:
        with tc.tile_pool(name="persist", bufs=1) as persist:
            # persistent SBUF tensors
            qt_sb = [persist.tile([P, LOCAL], bf16, tag=f"qt{h}", name=f"qt{h}") for h in range(HEADS)]
            avt_sb = [persist.tile([P, LOCAL], bf16, tag=f"avt{h}", name=f"avt{h}") for h in range(HEADS)]
            ones_sb = persist.tile([P, P], bf16, tag="ones")
            nc.vector.memset(ones_sb[:], 1.0)
            eps_sb = persist.tile([P, 1], fp32, tag="eps")
            nc.vector.memset(eps_sb[:], EPS)

            # ---------------- Phase 1: rmsnorm + transpose -> xT ----------
            with (
                tc.tile_pool(name="p1", bufs=2) as p1,
                tc.tile_pool(name="xt", bufs=1) as xtp,
                tc.tile_pool(name="stat", bufs=4) as stat,
            ):
                xT = [xtp.tile([P, LOCAL], bf16, tag=f"xt{m}", name=f"xt{m}") for m in range(MC)]
                for rt in range(RT):
                    x = p1.tile([P, DIM], fp32, tag="x")
                    nc.sync.dma_start(out=x[:], in_=toks[rt * P:(rt + 1) * P, :])
                    sq = p1.tile([P, DIM], fp32, tag="sq")
                    ssq = stat.tile([P, 1], fp32, tag="ssq")
                    nc.scalar.activation(
                        sq[:], x[:], mybir.ActivationFunctionType.Square,
                        accum_out=ssq[:],
                    )
                    std = stat.tile([P, 1], fp32, tag="std")
                    nc.scalar.activation(
                        std[:], ssq[:], mybir.ActivationFunctionType.Sqrt,
                        bias=eps_sb[:], scale=1.0 / DIM,
                    )
                    rstd = stat.tile([P, 1], fp32, tag="rstd")
                    nc.vector.reciprocal(rstd[:], std[:])
                    xn = p1.tile([P, DIM], bf16, tag="xn")
                    nc.scalar.activation(
                        xn[:], x[:], mybir.ActivationFunctionType.Copy,
                        scale=rstd[:],
                    )
                    for m in range(MC):
                        nc.sync.dma_start_transpose(
                            out=xT[m][:, rt * P:(rt + 1) * P],
                            in_=xn[:, m * P:(m + 1) * P],
                        )

                # ---------------- Phase 2: k^T and v projections ----------
                with (
                    tc.tile_pool(name="p2", bufs=2) as p2,
                    tc.tile_pool(name="psA", bufs=2, space="PSUM") as psum_pool,
                ):
                    # k^T: stationary = wk chunks, moving = xT
                    for h in range(HEADS):
                        wk_sb = p2.tile([P, MC, P], bf16, tag="wk")
                        nc.sync.dma_start(out=wk_sb[:], in_=wk[h])
                        for q in range(QH):
                            ps = psum_pool.tile([P, FD], fp32, tag="pp")
                            for m in range(MC):
                                nc.tensor.matmul(
                                    ps[:], wk_sb[:, m, :],
                                    xT[m][:, q * FD:(q + 1) * FD],
                                    start=(m == 0), stop=(m == MC - 1),
                                )
                            stg = p2.tile([P, FD], bf16, tag="stg")
                            nc.vector.tensor_copy(stg[:], ps[:])
                            nc.sync.dma_start(
                                out=kl[h][:, q * FD:(q + 1) * FD], in_=stg[:]
                            )
                    # v: stationary = xT chunks, moving = wv[mc, oc] tiles
                    for oc in range(OC):
                        wv_sb = p2.tile([P, MC, FD], bf16, tag="wv")
                        nc.sync.dma_start(
                            out=wv_sb[:],
                            in_=wv.ap().rearrange("(mc p) d -> p mc d", p=P)[
                                :, :, oc * FD:(oc + 1) * FD
                            ],
                        )
                        for rt in range(RT):
                            ps = psum_pool.tile([P, FD], fp32, tag="pp")
                            for m in range(MC):
                                nc.tensor.matmul(
                                    ps[:], xT[m][:, rt * P:(rt + 1) * P],
                                    wv_sb[:, m, :],
                                    start=(m == 0), stop=(m == MC - 1),
                                )
                            stg = p2.tile([P, FD], bf16, tag="stg")
                            nc.vector.tensor_copy(stg[:], ps[:])
                            nc.sync.dma_start(
                                out=vl[rt][:, oc * FD:(oc + 1) * FD], in_=stg[:]
                            )

                    # pair-wise AllGather of k^T and v
                    nc.gpsimd.collective_compute(
                        "AllGather", mybir.AluOpType.bypass,
                        replica_groups=RG,
                        ins=[kl.ap().opt()], outs=[kg.ap().opt()],
                    )
                    nc.gpsimd.collective_compute(
                        "AllGather", mybir.AluOpType.bypass,
                        replica_groups=RG,
                        ins=[vl.ap().opt()], outs=[vg.ap().opt()],
                    )

                    # ---------------- Phase 3: q^T projection --------------
                    for h in range(HEADS):
                        wq_sb = p2.tile([P, MC, P], bf16, tag="wk")
                        nc.sync.dma_start(out=wq_sb[:], in_=wq[h])
                        for q in range(QH):
                            ps = psum_pool.tile([P, FD], fp32, tag="pp")
                            for m in range(MC):
                                nc.tensor.matmul(
                                    ps[:], wq_sb[:, m, :],
                                    xT[m][:, q * FD:(q + 1) * FD],
                                    start=(m == 0), stop=(m == MC - 1),
                                )
                            nc.vector.tensor_copy(
                                qt_sb[h][:, q * FD:(q + 1) * FD], ps[:]
                            )

            # ---------------- Phase 4: attention ---------------------------
            with (
                tc.tile_pool(name="attn", bufs=2) as ap_,
                tc.tile_pool(name="psB", bufs=2, space="PSUM") as psum_pool,
            ):
                for h in range(HEADS):
                    kt_sb = ap_.tile([P, N], bf16, tag="kt")
                    for b in range(2):
                        nc.sync.dma_start(
                            out=kt_sb[:, b * LOCAL:(b + 1) * LOCAL], in_=kg[b, h]
                        )
                    v_sb = ap_.tile([P, KCHUNKS, P], bf16, tag="vt")
                    for b in range(2):
                        for rt in range(RT):
                            nc.sync.dma_start(
                                out=v_sb[:, b * RT + rt, :],
                                in_=vg[b, rt][:, h * DHEAD:(h + 1) * DHEAD],
                            )
                    for q in range(QH):
                        av = psum_pool.tile([P, FD], fp32, tag="av")
                        rs = psum_pool.tile([P, FD], fp32, tag="rs")
                        for kc in range(KCHUNKS):
                            sim = psum_pool.tile([P, FD], fp32, tag="sim")
                            nc.tensor.matmul(
                                sim[:], kt_sb[:, kc * P:(kc + 1) * P],
                                qt_sb[h][:, q * FD:(q + 1) * FD],
                                start=True, stop=True,
                            )
                            pT = ap_.tile([P, FD], bf16, tag="pT")
                            nc.scalar.activation(
                                pT[:], sim[:], mybir.ActivationFunctionType.Exp
                            )
                            nc.tensor.matmul(
                                av[:], v_sb[:, kc, :], pT[:],
                                start=(kc == 0), stop=(kc == KCHUNKS - 1),
                            )
                            nc.tensor.matmul(
                                rs[:], ones_sb[:], pT[:],
                                start=(kc == 0), stop=(kc == KCHUNKS - 1),
                            )
                        recip = ap_.tile([P, FD], fp32, tag="recip")
                        nc.vector.reciprocal(recip[:], rs[:])
                        nc.vector.tensor_mul(
                            avt_sb[h][:, q * FD:(q + 1) * FD], av[:], recip[:]
                        )

            # ---------------- Phase 5: output projection -------------------
            with (
                tc.tile_pool(name="proj", bufs=2) as pr,
                tc.tile_pool(name="psC", bufs=2, space="PSUM") as psum_pool,
            ):
                for oc in range(OC):
                    wo_sb = pr.tile([P, HEADS, FD], bf16, tag="wo")
                    nc.sync.dma_start(
                        out=wo_sb[:],
                        in_=wo.ap().rearrange("(hh p) d -> p hh d", p=P)[
                            :, :, oc * FD:(oc + 1) * FD
                        ],
                    )
                    for qt in range(RT):
                        ps = psum_pool.tile([P, FD], fp32, tag="pp5")
                        for h in range(HEADS):
                            nc.tensor.matmul(
                                ps[:], avt_sb[h][:, qt * P:(qt + 1) * P],
                                wo_sb[:, h, :],
                                start=(h == 0), stop=(h == HEADS - 1),
                            )
                        ostg = pr.tile([P, FD], fp32, tag="ostg")
                        nc.vector.tensor_copy(ostg[:], ps[:])
                        nc.sync.dma_start(
                            out=out[qt * P:(qt + 1) * P, oc * FD:(oc + 1) * FD],
                            in_=ostg[:],
                        )

    nc.compile()
    return nc


def _get_nc():
    global _CACHED_NC
    if _CACHED_NC is None:
        _CACHED_NC = build()
    return _CACHED_NC


def _make_in_maps(tokens, norm_weight, w_q, w_kv, w_out):
    tokens = np.asarray(tokens, dtype=np.float32)
    norm_weight = np.asarray(norm_weight, dtype=np.float32)
    w_q = np.asarray(w_q, dtype=np.float32)
    w_kv = np.asarray(w_kv, dtype=np.float32)
    w_out = np.asarray(w_out, dtype=np.float32)

    wq_eff = (w_q * norm_weight[:, None]) * (DHEAD ** -0.5)
    wk_eff = w_kv[:, :DIM] * norm_weight[:, None]
    wv_eff = w_kv[:, DIM:] * norm_weight[:, None]

    def pack_T(w):  # [DIM, DIM] -> [h, p, mc, d]
        t = w.reshape(MC, P, HEADS, DHEAD)
        return np.ascontiguousarray(t.transpose(2, 1, 0, 3)).astype(BF16)

    wq_p = pack_T(wq_eff)
    wk_p = pack_T(wk_eff)
    wv_b = wv_eff.astype(BF16)
    wo_b = w_out.astype(BF16)

    in_maps = []
    for c in range(NCORES):
        bi, hi = c // 2, c % 2
        tk = np.ascontiguousarray(tokens[bi, hi * LOCAL:(hi + 1) * LOCAL])
        in_maps.append(
            {"tokens": tk, "wq": wq_p, "wk": wk_p, "wv": wv_b, "wo": wo_b}
        )
    return in_maps


def _assemble(results):
    out = np.empty((B, N, DIM), np.float32)
    for c in range(NCORES):
        bi, hi = c // 2, c % 2
        out[bi, hi * LOCAL:(hi + 1) * LOCAL] = results[c]["out"]
    return out


def run(trace=False, tmpdir=None, **inputs):
    from concourse.bass_utils import run_bass_kernel_spmd

    nc = _get_nc()
    in_maps = _make_in_maps(**inputs)
    res = run_bass_kernel_spmd(
        nc, in_maps, core_ids=list(range(NCORES)), trace=trace, tmpdir=tmpdir
    )
    return _assemble(res.results), res


def kernel(**inputs):
    out, _ = run(trace=False, **inputs)
    return out


# revision 6
# speedup vs baseline: 1.0542x; 1.0542x over previous
"""Distributed Trainium2 Bass kernel: RMSNorm + 16-head attention + out-proj.

Problem (hardcoded): tokens [4, 2048, 2048] f32, DIM=2048, HEADS=16, DHEAD=128.
Sharding: 8 cores = 4 batches x 2 sequence halves. Each core owns 1024 query
rows of one batch; K/V for the full 2048 keys arrive via a pair-wise AllGather
(cores 2i and 2i+1 hold the two halves of batch i). All matmuls run in bf16
with fp32 PSUM accumulation; rmsnorm is computed in fp32.

Host-side folding: norm_weight and the q-scale (dhead^-0.5) are folded into
the projection weight matrices, so the device kernel only computes
x * rsqrt(mean(x^2) + eps) for the norm.

Queue discipline: weight/bounce DMAs go on nc.sync; the xbar transposes and
the gather-dependent attention loads go on nc.scalar so a wait on the
collective semaphore never blocks weight prefetch.
"""

import os
import sys

for p in ("/opt/trn_rl_repo", "/root/.axon_site/_ro/trn_rl_repo"):
    if os.path.isdir(p) and p not in sys.path:
        sys.path.append(p)

import numpy as np
import ml_dtypes

BF16 = ml_dtypes.bfloat16

DIM = 2048
HEADS = 16
DHEAD = 128
B = 4
N = 2048
NCORES = 8
LOCAL = N // 2          # 1024 rows per core
P = 128                 # partitions
MC = DIM // P           # 16 model chunks
RT = LOCAL // P         # 8 row tiles
QH = 2                  # qrow halves of 512
FD = 512                # moving free dim
KCHUNKS = N // P        # 16 key chunks (over both blocks)
OC = DIM // FD          # 4 out-col chunks
EPS = 1.1920929e-07
KL = HEADS * P * LOCAL  # flat elems of the k^T shard (== RT*P*DIM)

_CACHED_NC = None


def build():
    from concourse import bacc, tile, mybir

    fp32 = mybir.dt.float32
    bf16 = mybir.dt.bfloat16

    nc = bacc.Bacc(
        "TRN2", target_bir_lowering=False, debug=False, num_devices=NCORES
    )

    toks = nc.dram_tensor("tokens", [LOCAL, DIM], fp32, kind="ExternalInput")
    wq = nc.dram_tensor("wq", [HEADS, P, MC, P], bf16, kind="ExternalInput")
    wk = nc.dram_tensor("wk", [HEADS, P, MC, P], bf16, kind="ExternalInput")
    wv = nc.dram_tensor("wv", [DIM, DIM], bf16, kind="ExternalInput")
    wo = nc.dram_tensor("wo", [DIM, DIM], bf16, kind="ExternalInput")
    out = nc.dram_tensor("out", [LOCAL, DIM], fp32, kind="ExternalOutput")

    # internal DRAM scratch
    xn_dram = nc.dram_tensor("xn_dram", [MC, LOCAL, P], bf16)
    kv_l = nc.dram_tensor("kv_l", [2 * KL], bf16)        # k^T ++ v, local shard
    kv_g = nc.dram_tensor("kv_g", [2, 2 * KL], bf16)     # pair-gathered

    def kl_view(flat):   # -> [HEADS, P, LOCAL]
        return flat[0:KL].rearrange("(h p k) -> h p k", h=HEADS, p=P)

    def vl_view(flat):   # -> [RT, P, DIM]
        return flat[KL:2 * KL].rearrange("(r p d) -> r p d", r=RT, p=P)

    klv, vlv = kl_view(kv_l.ap()), vl_view(kv_l.ap())
    kgv = [kl_view(kv_g[b]) for b in range(2)]
    vgv = [vl_view(kv_g[b]) for b in range(2)]

    RG = [[0, 1], [2, 3], [4, 5], [6, 7]]

    with tile.TileContext(nc) as tc:
        with tc.tile_pool(name="persist", bufs=1) as persist:
            qt_sb = [persist.tile([P, LOCAL], bf16, tag=f"qt{h}", name=f"qt{h}")
                     for h in range(HEADS)]
            avt_sb = [persist.tile([P, LOCAL], bf16, tag=f"avt{h}", name=f"avt{h}")
                      for h in range(HEADS)]
            ones_sb = persist.tile([P, P], bf16, tag="ones")
            nc.vector.memset(ones_sb[:], 1.0)
            eps_sb = persist.tile([P, 1], fp32, tag="eps")
            nc.vector.memset(eps_sb[:], EPS)

            # ---------------- Phase 1: rmsnorm -> xn_dram -> xT ------------
            with (
                tc.tile_pool(name="p1", bufs=2) as p1,
                tc.tile_pool(name="xt", bufs=1) as xtp,
                tc.tile_pool(name="stat", bufs=4) as stat,
            ):
                xT = [xtp.tile([P, LOCAL], bf16, tag=f"xt{m}", name=f"xt{m}")
                      for m in range(MC)]
                for rt in range(RT):
                    x = p1.tile([P, DIM], fp32, tag="x")
                    nc.sync.dma_start(out=x[:], in_=toks[rt * P:(rt + 1) * P, :])
                    sq = p1.tile([P, DIM], fp32, tag="sq")
                    ssq = stat.tile([P, 1], fp32, tag="ssq")
                    nc.scalar.activation(
                        sq[:], x[:], mybir.ActivationFunctionType.Square,
                        accum_out=ssq[:],
                    )
                    std = stat.tile([P, 1], fp32, tag="std")
                    nc.scalar.activation(
                        std[:], ssq[:], mybir.ActivationFunctionType.Sqrt,
                        bias=eps_sb[:], scale=1.0 / DIM,
                    )
                    rstd = stat.tile([P, 1], fp32, tag="rstd")
                    nc.vector.reciprocal(rstd[:], std[:])
                    xn = p1.tile([P, DIM], bf16, tag="xn")
                    nc.scalar.activation(
                        xn[:], x[:], mybir.ActivationFunctionType.Copy,
                        scale=rstd[:],
                    )
                    # chunk-major bounce to DRAM so each transpose source is
                    # one fully-contiguous [LOCAL, P] block
                    nc.sync.dma_start(
                        out=xn_dram[:, rt * P:(rt + 1) * P, :].rearrange(
                            "m r p -> r m p"
                        ),
                        in_=xn[:].rearrange("r (m p) -> r m p", p=P),
                    )
                for m in range(MC):
                    nc.scalar.dma_start_transpose(out=xT[m][:], in_=xn_dram[m])

                # ---------------- Phase 2: k^T and v projections ----------
                with (
                    tc.tile_pool(name="p2", bufs=2) as p2,
                    tc.tile_pool(name="psA", bufs=2, space="PSUM") as psA,
                ):
                    # k^T: stationary = wk chunks, moving = xT
                    for h in range(HEADS):
                        wk_sb = p2.tile([P, MC, P], bf16, tag="wk")
                        nc.sync.dma_start(out=wk_sb[:], in_=wk[h])
                        for q in range(QH):
                            ps = psA.tile([P, FD], fp32, tag="pp")
                            for m in range(MC):
                                nc.tensor.matmul(
                                    ps[:], wk_sb[:, m, :],
                                    xT[m][:, q * FD:(q + 1) * FD],
                                    start=(m == 0), stop=(m == MC - 1),
                                )
                            stg = p2.tile([P, FD], bf16, tag="stg")
                            nc.vector.tensor_copy(stg[:], ps[:])
                            nc.sync.dma_start(
                                out=klv[h][:, q * FD:(q + 1) * FD], in_=stg[:]
                            )
                    # v: stationary = xT chunks, moving = wv[mc, oc] tiles
                    for oc in range(OC):
                        wv_sb = p2.tile([P, MC, FD], bf16, tag="wv")
                        nc.sync.dma_start(
                            out=wv_sb[:],
                            in_=wv.ap().rearrange("(mc p) d -> p mc d", p=P)[
                                :, :, oc * FD:(oc + 1) * FD
                            ],
                        )
                        for rt in range(RT):
                            ps = psA.tile([P, FD], fp32, tag="pp")
                            for m in range(MC):
                                nc.tensor.matmul(
                                    ps[:], xT[m][:, rt * P:(rt + 1) * P],
                                    wv_sb[:, m, :],
                                    start=(m == 0), stop=(m == MC - 1),
                                )
                            stg = p2.tile([P, FD], bf16, tag="stg")
                            nc.vector.tensor_copy(stg[:], ps[:])
                            nc.sync.dma_start(
                                out=vlv[rt][:, oc * FD:(oc + 1) * FD], in_=stg[:]
                            )

                    # one pair-wise AllGather for k^T ++ v
                    nc.gpsimd.collective_compute(
                        "AllGather", mybir.AluOpType.bypass,
                        replica_groups=RG,
                        ins=[kv_l.ap().opt()], outs=[kv_g.ap().opt()],
                    )

                    # ---------------- Phase 3: q^T projection --------------
                    for h in range(HEADS):
                        wq_sb = p2.tile([P, MC, P], bf16, tag="wk")
                        nc.sync.dma_start(out=wq_sb[:], in_=wq[h])
                        for q in range(QH):
                            ps = psA.tile([P, FD], fp32, tag="pp")
                            for m in range(MC):
                                nc.tensor.matmul(
                                    ps[:], wq_sb[:, m, :],
                                    xT[m][:, q * FD:(q + 1) * FD],
                                    start=(m == 0), stop=(m == MC - 1),
                                )
                            nc.vector.tensor_copy(
                                qt_sb[h][:, q * FD:(q + 1) * FD], ps[:]
                            )

            # ---------------- Phase 4: attention ---------------------------
            with (
                tc.tile_pool(name="attn", bufs=2) as ap_,
                tc.tile_pool(name="psB", bufs=2, space="PSUM") as psB,
            ):
                for h in range(HEADS):
                    kt_sb = ap_.tile([P, N], bf16, tag="kt")
                    for b in range(2):
                        nc.scalar.dma_start(
                            out=kt_sb[:, b * LOCAL:(b + 1) * LOCAL], in_=kgv[b][h]
                        )
                    v_sb = ap_.tile([P, KCHUNKS, P], bf16, tag="vt")
                    for b in range(2):
                        for rt in range(RT):
                            nc.scalar.dma_start(
                                out=v_sb[:, b * RT + rt, :],
                                in_=vgv[b][rt][:, h * DHEAD:(h + 1) * DHEAD],
                            )
                    for q in range(QH):
                        av = psB.tile([P, FD], fp32, tag="av")
                        rs = psB.tile([P, FD], fp32, tag="rs")
                        for kp in range(KCHUNKS // 2):
                            sim = psB.tile([P, 2, FD], fp32, tag="sim")
                            for j in range(2):
                                kc = kp * 2 + j
                                nc.tensor.matmul(
                                    sim[:, j, :], kt_sb[:, kc * P:(kc + 1) * P],
                                    qt_sb[h][:, q * FD:(q + 1) * FD],
                                    start=True, stop=True,
                                )
                            pT = ap_.tile([P, 2, FD], bf16, tag="pT")
                            nc.scalar.activation(
                                pT[:], sim[:], mybir.ActivationFunctionType.Exp
                            )
                            for j in range(2):
                                kc = kp * 2 + j
                                nc.tensor.matmul(
                                    av[:], v_sb[:, kc, :], pT[:, j, :],
                                    start=(kc == 0), stop=(kc == KCHUNKS - 1),
                                )
                            for j in range(2):
                                kc = kp * 2 + j
                                nc.tensor.matmul(
                                    rs[:], ones_sb[:], pT[:, j, :],
                                    start=(kc == 0), stop=(kc == KCHUNKS - 1),
                                )
                        rc1 = ap_.tile([1, FD], fp32, tag="rc1")
                        nc.vector.reciprocal(rc1[:], rs[0:1, :])
                        rcb = ap_.tile([P, FD], fp32, tag="rcb")
                        nc.gpsimd.partition_broadcast(rcb[:], rc1[:])
                        nc.vector.tensor_mul(
                            avt_sb[h][:, q * FD:(q + 1) * FD], av[:], rcb[:]
                        )

            # ---------------- Phase 5: output projection -------------------
            with (
                tc.tile_pool(name="proj", bufs=2) as pr,
                tc.tile_pool(name="psC", bufs=2, space="PSUM") as psC,
            ):
                wo_sb = pr.tile([P, MC, DIM], bf16, tag="wo", bufs=1)
                nc.sync.dma_start(
                    out=wo_sb[:],
                    in_=wo.ap().rearrange("(hh p) d -> p hh d", p=P),
                )
                for qt in range(RT):
                    ps = psC.tile([P, OC, FD], fp32, tag="po")
                    for h in range(HEADS):
                        for oc in range(OC):
                            nc.tensor.matmul(
                                ps[:, oc, :], avt_sb[h][:, qt * P:(qt + 1) * P],
                                wo_sb[:, h, oc * FD:(oc + 1) * FD],
                                start=(h == 0), stop=(h == HEADS - 1),
                            )
                    for oc in range(OC):
                        ostg = pr.tile([P, FD], fp32, tag="ostg")
                        nc.vector.tensor_copy(ostg[:], ps[:, oc, :])
                        nc.sync.dma_start(
                            out=out[qt * P:(qt + 1) * P, oc * FD:(oc + 1) * FD],
                            in_=ostg[:],
                        )

    nc.compile()
    return nc


def _get_nc():
    global _CACHED_NC
    if _CACHED_NC is None:
        _CACHED_NC = build()
    return _CACHED_NC


def _make_in_maps(tokens, norm_weight, w_q, w_kv, w_out):
    tokens = np.asarray(tokens, dtype=np.float32)
    norm_weight = np.asarray(norm_weight, dtype=np.float32)
    w_q = np.asarray(w_q, dtype=np.float32)
    w_kv = np.asarray(w_kv, dtype=np.float32)
    w_out = np.asarray(w_out, dtype=np.float32)

    wq_eff = (w_q * norm_weight[:, None]) * (DHEAD ** -0.5)
    wk_eff = w_kv[:, :DIM] * norm_weight[:, None]
    wv_eff = w_kv[:, DIM:] * norm_weight[:, None]

    def pack_T(w):  # [DIM, DIM] -> [h, p, mc, d]
        t = w.reshape(MC, P, HEADS, DHEAD)
        return np.ascontiguousarray(t.transpose(2, 1, 0, 3)).astype(BF16)

    wq_p = pack_T(wq_eff)
    wk_p = pack_T(wk_eff)
    wv_b = wv_eff.astype(BF16)
    wo_b = w_out.astype(BF16)

    in_maps = []
    for c in range(NCORES):
        bi, hi = c // 2, c % 2
        tk = np.ascontiguousarray(tokens[bi, hi * LOCAL:(hi + 1) * LOCAL])
        in_maps.append(
            {"tokens": tk, "wq": wq_p, "wk": wk_p, "wv": wv_b, "wo": wo_b}
        )
    return in_maps


def _assemble(results):
    out = np.empty((B, N, DIM), np.float32)
    for c in range(NCORES):
        bi, hi = c // 2, c % 2
        out[bi, hi * LOCAL:(hi + 1) * LOCAL] = results[c]["out"]
    return out


def run(trace=False, tmpdir=None, **inputs):
    from concourse.bass_utils import run_bass_kernel_spmd

    nc = _get_nc()
    in_maps = _make_in_maps(**inputs)
    res = run_bass_kernel_spmd(
        nc, in_maps, core_ids=list(range(NCORES)), trace=trace, tmpdir=tmpdir
    )
    return _assemble(res.results), res


def kernel(**inputs):
    out, _ = run(trace=False, **inputs)
    return out


# revision 13
# speedup vs baseline: 1.1417x; 1.0830x over previous
"""Distributed Trainium2 Bass kernel: RMSNorm + 16-head attention + out-proj.

Problem (hardcoded): tokens [4, 2048, 2048] f32, DIM=2048, HEADS=16, DHEAD=128.
Sharding: 8 cores = 4 batches x 2 sequence halves. Each core owns 1024 query
rows of one batch; K/V for the full 2048 keys arrive via pair-wise AllGathers
(cores 2i and 2i+1 hold the two halves of batch i). All matmuls run in bf16
with fp32 PSUM accumulation; rmsnorm is computed in fp32.

Host-side folding: norm_weight and the q-scale (dhead^-0.5) are folded into
the projection weight matrices, so the device kernel only computes
x * rsqrt(mean(x^2) + eps) for the norm.

Queue discipline: weight/bounce DMAs go on nc.sync (HWDGE); the xbar
transposes go on nc.scalar; the gather-dependent attention loads go on
nc.gpsimd (SWDGE rings) so a wait on the collective semaphore never blocks
weight prefetch through shared HWDGE ring flow-control.
"""

import os
import sys

for p in ("/opt/trn_rl_repo", "/root/.axon_site/_ro/trn_rl_repo"):
    if os.path.isdir(p) and p not in sys.path:
        sys.path.append(p)

import numpy as np
import ml_dtypes

BF16 = ml_dtypes.bfloat16

DIM = 2048
HEADS = 16
DHEAD = 128
B = 4
N = 2048
NCORES = 8
LOCAL = N // 2          # 1024 rows per core
P = 128                 # partitions
MC = DIM // P           # 16 model chunks
RT = LOCAL // P         # 8 row tiles
QH = 2                  # qrow halves of 512
FD = 512                # moving free dim
KCHUNKS = N // P        # 16 key chunks (over both blocks)
OC = DIM // FD          # 4 out-col chunks
EPS = 1.1920929e-07
KL = HEADS * P * LOCAL  # flat elems of the k^T shard (== RT*P*DIM)

_CACHED_NC = None


def build():
    from concourse import bacc, tile, mybir

    fp32 = mybir.dt.float32
    bf16 = mybir.dt.bfloat16

    nc = bacc.Bacc(
        "TRN2", target_bir_lowering=False, debug=False, num_devices=NCORES
    )

    toks = nc.dram_tensor("tokens", [LOCAL, DIM], fp32, kind="ExternalInput")
    wq = nc.dram_tensor("wq", [HEADS, P, MC, P], bf16, kind="ExternalInput")
    wk = nc.dram_tensor("wk", [HEADS, P, MC, P], bf16, kind="ExternalInput")
    wv = nc.dram_tensor("wv", [DIM, DIM], bf16, kind="ExternalInput")
    wo = nc.dram_tensor("wo", [DIM, DIM], bf16, kind="ExternalInput")
    out = nc.dram_tensor("out", [LOCAL, DIM], fp32, kind="ExternalOutput")

    # internal DRAM scratch
    xn_dram = nc.dram_tensor("xn_dram", [MC, LOCAL, P], bf16)
    kl_d = nc.dram_tensor("kl_d", [HEADS, P, LOCAL], bf16)   # local k^T
    vl_d = nc.dram_tensor("vl_d", [RT, P, DIM], bf16)        # local v
    kg_d = nc.dram_tensor("kg_d", [2, HEADS, P, LOCAL], bf16)
    vg_d = nc.dram_tensor("vg_d", [2, RT, P, DIM], bf16)

    klv, vlv = kl_d.ap(), vl_d.ap()
    kgv = [kg_d[b] for b in range(2)]
    vgv = [vg_d[b] for b in range(2)]

    RG = [[0, 1], [2, 3], [4, 5], [6, 7]]

    with tile.TileContext(nc) as tc:
      with tc.tile_pool(name="persist", bufs=1) as persist:
        qt_sb = [persist.tile([P, LOCAL], bf16, tag=f"qt{h}", name=f"qt{h}")
                 for h in range(HEADS)]
        avt_sb = [persist.tile([P, LOCAL], bf16, tag=f"avt{h}", name=f"avt{h}")
                  for h in range(HEADS)]
        ones_sb = persist.tile([P, P], bf16, tag="ones")
        nc.vector.memset(ones_sb[:], 1.0)
        eps_sb = persist.tile([P, 1], fp32, tag="eps")
        nc.vector.memset(eps_sb[:], EPS)

        # ---------------- Phase 1: rmsnorm -> xn_dram -> xT ----------------
        with tc.tile_pool(name="xt", bufs=1) as xtp:
            xT = [xtp.tile([P, LOCAL], bf16, tag=f"xt{m}", name=f"xt{m}")
                  for m in range(MC)]
            with (
                tc.tile_pool(name="p1", bufs=2) as p1,
                tc.tile_pool(name="stat", bufs=4) as stat,
            ):
                for rt in range(RT):
                    x = p1.tile([P, DIM], fp32, tag="x")
                    nc.sync.dma_start(out=x[:], in_=toks[rt * P:(rt + 1) * P, :])
                    sq = p1.tile([P, DIM], fp32, tag="sq")
                    ssq = stat.tile([P, 1], fp32, tag="ssq")
                    nc.scalar.activation(
                        sq[:], x[:], mybir.ActivationFunctionType.Square,
                        accum_out=ssq[:],
                    )
                    std = stat.tile([P, 1], fp32, tag="std")
                    nc.scalar.activation(
                        std[:], ssq[:], mybir.ActivationFunctionType.Sqrt,
                        bias=eps_sb[:], scale=1.0 / DIM,
                    )
                    rstd = stat.tile([P, 1], fp32, tag="rstd")
                    nc.vector.reciprocal(rstd[:], std[:])
                    xn = p1.tile([P, DIM], bf16, tag="xn")
                    nc.scalar.activation(
                        xn[:], x[:], mybir.ActivationFunctionType.Copy,
                        scale=rstd[:],
                    )
                    # chunk-major bounce to DRAM so each transpose source is
                    # one fully-contiguous [LOCAL, P] block
                    nc.sync.dma_start(
                        out=xn_dram[:, rt * P:(rt + 1) * P, :].rearrange(
                            "m r p -> r m p"
                        ),
                        in_=xn[:].rearrange("r (m p) -> r m p", p=P),
                    )
            for m in range(MC):
                nc.scalar.dma_start_transpose(out=xT[m][:], in_=xn_dram[m])

            # ---------------- Phase 2: k^T and v projections ---------------
            with (
                tc.tile_pool(name="p2", bufs=2) as p2,
                tc.tile_pool(name="psA", bufs=2, space="PSUM") as psA,
            ):
                wq_full = p2.tile([P, HEADS, MC, P], bf16, tag="wqf", bufs=1)
                nc.sync.dma_start(
                    out=wq_full[:], in_=wq.ap().rearrange("h p m d -> p h m d")
                )
                # k^T: stationary = wk chunks, moving = xT
                for h in range(HEADS):
                    wk_sb = p2.tile([P, MC, P], bf16, tag="wk")
                    nc.sync.dma_start(out=wk_sb[:], in_=wk[h])
                    for q in range(QH):
                        ps = psA.tile([P, FD], fp32, tag="pp")
                        for m in range(MC):
                            nc.tensor.matmul(
                                ps[:], wk_sb[:, m, :],
                                xT[m][:, q * FD:(q + 1) * FD],
                                start=(m == 0), stop=(m == MC - 1),
                            )
                        stg = p2.tile([P, FD], bf16, tag="stg")
                        nc.vector.tensor_copy(stg[:], ps[:])
                        nc.sync.dma_start(
                            out=klv[h][:, q * FD:(q + 1) * FD], in_=stg[:]
                        )
                # k-gather starts while v/q projections run
                nc.gpsimd.collective_compute(
                    "AllGather", mybir.AluOpType.bypass,
                    replica_groups=RG,
                    ins=[kl_d.ap().opt()],
                    outs=[kg_d.ap().opt()],
                )
                # v: stationary = xT chunks, moving = wv[mc, oc] tiles
                for oc in range(OC):
                    wv_sb = p2.tile([P, MC, FD], bf16, tag="wv")
                    nc.sync.dma_start(
                        out=wv_sb[:],
                        in_=wv.ap().rearrange("(mc p) d -> p mc d", p=P)[
                            :, :, oc * FD:(oc + 1) * FD
                        ],
                    )
                    for rt in range(RT):
                        ps = psA.tile([P, FD], fp32, tag="pp")
                        for m in range(MC):
                            nc.tensor.matmul(
                                ps[:], xT[m][:, rt * P:(rt + 1) * P],
                                wv_sb[:, m, :],
                                start=(m == 0), stop=(m == MC - 1),
                            )
                        stg = p2.tile([P, FD], bf16, tag="stg")
                        nc.vector.tensor_copy(stg[:], ps[:])
                        nc.sync.dma_start(
                            out=vlv[rt][:, oc * FD:(oc + 1) * FD], in_=stg[:]
                        )
                nc.gpsimd.collective_compute(
                    "AllGather", mybir.AluOpType.bypass,
                    replica_groups=RG,
                    ins=[vl_d.ap().opt()],
                    outs=[vg_d.ap().opt()],
                )

                # ---------------- Phase 3: q^T projection ------------------
                for h in range(HEADS):
                    for q in range(QH):
                        ps = psA.tile([P, FD], fp32, tag="pp")
                        for m in range(MC):
                            nc.tensor.matmul(
                                ps[:], wq_full[:, h, m, :],
                                xT[m][:, q * FD:(q + 1) * FD],
                                start=(m == 0), stop=(m == MC - 1),
                            )
                        nc.vector.tensor_copy(
                            qt_sb[h][:, q * FD:(q + 1) * FD], ps[:]
                        )

        # ---------------- Phase 4: attention -------------------------------
        with tc.tile_pool(name="proj", bufs=2) as pr:
            wo_sb = pr.tile([P, MC, DIM], bf16, tag="wo", bufs=1)
            nc.sync.dma_start(
                out=wo_sb[:],
                in_=wo.ap().rearrange("(hh p) d -> p hh d", p=P),
            )
            with (
                tc.tile_pool(name="attn", bufs=2) as ap_,
                tc.tile_pool(name="psB", bufs=2, space="PSUM") as psB,
            ):
                for h in range(HEADS):
                    kt_sb = ap_.tile([P, N], bf16, tag="kt")
                    for b in range(2):
                        nc.gpsimd.dma_start(
                            out=kt_sb[:, b * LOCAL:(b + 1) * LOCAL], in_=kgv[b][h]
                        )
                    v_sb = ap_.tile([P, KCHUNKS, P], bf16, tag="vt")
                    for b in range(2):
                        nc.gpsimd.dma_start(
                            out=v_sb[:, b * RT:(b + 1) * RT, :],
                            in_=vgv[b][:, :, h * DHEAD:(h + 1) * DHEAD].rearrange(
                                "r p d -> p r d"
                            ),
                        )
                    for q in range(QH):
                        av = psB.tile([P, FD], fp32, tag="av")
                        rs = psB.tile([P, FD], fp32, tag="rs")
                        for kp in range(KCHUNKS // 2):
                            sim = psB.tile([P, 2, FD], fp32, tag="sim")
                            for j in range(2):
                                kc = kp * 2 + j
                                nc.tensor.matmul(
                                    sim[:, j, :], kt_sb[:, kc * P:(kc + 1) * P],
                                    qt_sb[h][:, q * FD:(q + 1) * FD],
                                    start=True, stop=True,
                                )
                            pT = ap_.tile([P, 2, FD], bf16, tag="pT")
                            nc.scalar.activation(
                                pT[:], sim[:], mybir.ActivationFunctionType.Exp
                            )
                            for j in range(2):
                                kc = kp * 2 + j
                                nc.tensor.matmul(
                                    av[:], v_sb[:, kc, :], pT[:, j, :],
                                    start=(kc == 0), stop=(kc == KCHUNKS - 1),
                                )
                            for j in range(2):
                                kc = kp * 2 + j
                                nc.tensor.matmul(
                                    rs[:], ones_sb[:], pT[:, j, :],
                                    start=(kc == 0), stop=(kc == KCHUNKS - 1),
                                )
                        rc1 = ap_.tile([1, FD], fp32, tag="rc1")
                        nc.vector.reciprocal(rc1[:], rs[0:1, :])
                        rcb = ap_.tile([P, FD], fp32, tag="rcb")
                        nc.gpsimd.partition_broadcast(rcb[:], rc1[:])
                        nc.vector.tensor_mul(
                            avt_sb[h][:, q * FD:(q + 1) * FD], av[:], rcb[:]
                        )

            # ---------------- Phase 5: output projection -------------------
            with tc.tile_pool(name="psC", bufs=2, space="PSUM") as psC:
                for qt in range(RT):
                    ps = psC.tile([P, OC, FD], fp32, tag="po")
                    for h in range(HEADS):
                        for oc in range(OC):
                            nc.tensor.matmul(
                                ps[:, oc, :], avt_sb[h][:, qt * P:(qt + 1) * P],
                                wo_sb[:, h, oc * FD:(oc + 1) * FD],
                                start=(h == 0), stop=(h == HEADS - 1),
                            )
                    for oc in range(OC):
                        ostg = pr.tile([P, FD], fp32, tag="ostg")
                        nc.vector.tensor_copy(ostg[:], ps[:, oc, :])
                        nc.sync.dma_start(
                            out=out[qt * P:(qt + 1) * P, oc * FD:(oc + 1) * FD],
                            in_=ostg[:],
                        )

    nc.compile()
    return nc


def _get_nc():
    global _CACHED_NC
    if _CACHED_NC is None:
        _CACHED_NC = build()
    return _CACHED_NC


def _make_in_maps(tokens, norm_weight, w_q, w_kv, w_out):
    tokens = np.asarray(tokens, dtype=np.float32)
    norm_weight = np.asarray(norm_weight, dtype=np.float32)
    w_q = np.asarray(w_q, dtype=np.float32)
    w_kv = np.asarray(w_kv, dtype=np.float32)
    w_out = np.asarray(w_out, dtype=np.float32)

    wq_eff = (w_q * norm_weight[:, None]) * (DHEAD ** -0.5)
    wk_eff = w_kv[:, :DIM] * norm_weight[:, None]
    wv_eff = w_kv[:, DIM:] * norm_weight[:, None]

    def pack_T(w):  # [DIM, DIM] -> [h, p, mc, d]
        t = w.reshape(MC, P, HEADS, DHEAD)
        return np.ascontiguousarray(t.transpose(2, 1, 0, 3)).astype(BF16)

    wq_p = pack_T(wq_eff)
    wk_p = pack_T(wk_eff)
    wv_b = wv_eff.astype(BF16)
    wo_b = w_out.astype(BF16)

    in_maps = []
    for c in range(NCORES):
        bi, hi = c // 2, c % 2
        tk = np.ascontiguousarray(tokens[bi, hi * LOCAL:(hi + 1) * LOCAL])
        in_maps.append(
            {"tokens": tk, "wq": wq_p, "wk": wk_p, "wv": wv_b, "wo": wo_b}
        )
    return in_maps


def _assemble(results):
    out = np.empty((B, N, DIM), np.float32)
    for c in range(NCORES):
        bi, hi = c // 2, c % 2
        out[bi, hi * LOCAL:(hi + 1) * LOCAL] = results[c]["out"]
    return out


def run(trace=False, tmpdir=None, **inputs):
    from concourse.bass_utils import run_bass_kernel_spmd

    nc = _get_nc()
    in_maps = _make_in_maps(**inputs)
    res = run_bass_kernel_spmd(
        nc, in_maps, core_ids=list(range(NCORES)), trace=trace, tmpdir=tmpdir
    )
    return _assemble(res.results), res


def kernel(**inputs):
    out, _ = run(trace=False, **inputs)
    return out
